# revision 17
# baseline (speedup 1.0000x reference)
"""Trainium2 Bass kernel for nn_Attention_28604482191653.

Reference computation (B=4, S=2048, D=1024, H=4096, fp32):
    Q = x@Wq.T+bq; K = x@Wk.T+bk; V = x@Wv.T+bv     (per batch b)
    Q,K l2-normalized along features; sim = Q@K.T; attn = softmax(sim)
    out = attn@V; mlp: relu(out@W1.T+b1) -> relu(@W2.T+b2) -> @W3.T+b3

Sharding: 8 cores = (batch b, sequence half h); core c handles b=c//2,
query rows [h*1024, (h+1)*1024). K/V are recomputed per core pair (no
collectives). The host rotates x[b]'s rows so each core's own query rows
are always columns [0, 1024) of xT — softmax/PV are invariant to key
order, so K/V just use the rotated order consistently.

On-chip activations are feature-major ("T" = [features, rows]) so biases
and softmax scales land on the partition dimension; matmul dtype is
float32r (bf16-class PE rate, ~tf32 accuracy, fp32 PSUM accumulate).

Phases are ordered K -> Q -> V -> attention -> MLP so that each
normalization chain (1/||row|| via ones-vector matmuls + sqrt/recip +
DMA partition-broadcast, no PE instructions) hides under the next
matmul phase. DMA triggers alternate between the SP ("sync") HW-DGE
queue and the GpSimd SW-DGE queue; ACT-issued DMAs are avoided where
the ACT stream is busy (triggers are serialized with compute per
engine).

L2-normalize eps: reference uses max(||row||, 1e-12); row norms here are
~sqrt(D) >> eps with probability 1, so plain rsqrt matches.
"""

import numpy as np

B, S, D, H = 4, 2048, 1024, 4096
P = 128
NS = 512          # matmul moving-dim slice
QROWS = S // 2    # query rows per core
N_CORES = 8

_BUILT = None
_LAST_INSTS = None


def _build():
    import concourse.bass as bass
    import concourse.tile as tile
    from concourse import bacc, mybir

    F32 = mybir.dt.float32
    F32R = mybir.dt.float32r
    ALU = mybir.AluOpType
    AF = mybir.ActivationFunctionType

    nc = bacc.Bacc("TRN2", target_bir_lowering=False, debug=False)

    # ---- I/O ----
    xT = nc.dram_tensor("xT", [D, S], F32R, kind="ExternalInput")
    wqT = nc.dram_tensor("wqT", [D, D], F32R, kind="ExternalInput")
    wkT = nc.dram_tensor("wkT", [D, D], F32R, kind="ExternalInput")
    wvT = nc.dram_tensor("wvT", [D, D], F32R, kind="ExternalInput")
    w1T = nc.dram_tensor("w1T", [D, D], F32R, kind="ExternalInput")
    w2T = nc.dram_tensor("w2T", [D, H], F32R, kind="ExternalInput")
    w3T = nc.dram_tensor("w3T", [H, D], F32R, kind="ExternalInput")
    bq_col = nc.dram_tensor("bq_col", [P, D // P], F32, kind="ExternalInput")
    bk_col = nc.dram_tensor("bk_col", [P, D // P], F32, kind="ExternalInput")
    b1_col = nc.dram_tensor("b1_col", [P, D // P], F32, kind="ExternalInput")
    b2_col = nc.dram_tensor("b2_col", [P, H // P], F32, kind="ExternalInput")
    bv_row = nc.dram_tensor("bv_row", [1, D], F32, kind="ExternalInput")
    b3_row = nc.dram_tensor("b3_row", [1, D], F32R, kind="ExternalInput")
    ones_col = nc.dram_tensor("ones_col", [P, 1], F32R, kind="ExternalInput")
    ones_row = nc.dram_tensor("ones_row", [1, P], F32R, kind="ExternalInput")
    out = nc.dram_tensor("out", [QROWS, D], F32, kind="ExternalOutput")

    def bcast_ap(sliced, n):
        """AP replicating a [1, n] DRAM slice across all 128 partitions."""
        return bass.AP(tensor=sliced.tensor, offset=sliced.offset,
                       ap=[[0, P], [1, n]])

    DK = D // P    # 8 feature tiles of d_model
    SK = S // P    # 16 key-position tiles
    HK = H // P    # 32 hidden tiles
    QT8 = QROWS // P
    QS2 = QROWS // NS
    VG = 4         # V row-tiles per psum group

    with tile.TileContext(nc, pool_alloc_mode="queue") as tc:
        with tc.tile_pool(name="dram", bufs=1, space="DRAM") as dram, \
             tc.tile_pool(name="const", bufs=1) as const:
            v_scr = dram.tile([S, D], F32R)
            ot_scr = [dram.tile([D, NS], F32R, name=f"ot_scr{i}")
                      for i in range(QS2)]
            rk_scr = dram.tile([1, S], F32)
            rq_scr = dram.tile([1, QROWS], F32R)
            rs_scr = dram.tile([1, QROWS], F32R)

            ones_sb = const.tile([P, 1], F32R)
            nc.sync.dma_start(out=ones_sb[:], in_=ones_col[:, :])
            onesr_sb = const.tile([1, P], F32R)
            nc.sync.dma_start(out=onesr_sb[:], in_=ones_row[:, :])
            bqc = const.tile([P, DK], F32)
            nc.sync.dma_start(out=bqc[:], in_=bq_col[:, :])
            bkc = const.tile([P, DK], F32)
            nc.sync.dma_start(out=bkc[:], in_=bk_col[:, :])
            b1c = const.tile([P, DK], F32)
            nc.sync.dma_start(out=b1c[:], in_=b1_col[:, :])
            b2c = const.tile([P, HK], F32)
            nc.sync.dma_start(out=b2c[:], in_=b2_col[:, :])
            rk_col = const.tile([P, SK], F32)

            qtp = tc.alloc_tile_pool(name="qt", bufs=1)
            qt_sb = qtp.tile([P, DK, QROWS], F32R)
            ktp = tc.alloc_tile_pool(name="kt", bufs=1)
            kt_sb = ktp.tile([P, DK, S], F32R)
            xtap = tc.alloc_tile_pool(name="xta", bufs=1)
            xtbp = tc.alloc_tile_pool(name="xtb", bufs=1)
            xta_sb = xtap.tile([P, DK // 2, S], F32R)
            xtb_sb = xtbp.tile([P, DK // 2, S], F32R)

            def xts(kk, sl):
                if kk < DK // 2:
                    return xta_sb[:, kk, sl]
                return xtb_sb[:, kk - DK // 2, sl]

            for nn in range(S // NS):
                for kk in range(DK):
                    eng = nc.sync if kk % 2 == 0 else nc.gpsimd
                    eng.dma_start(
                        out=xts(kk, slice(nn * NS, (nn + 1) * NS)),
                        in_=xT[kk * P:(kk + 1) * P, nn * NS:(nn + 1) * NS])

            # ======== K projection (feature-major, resident) ========
            with tc.tile_pool(name="wk", bufs=4) as wkp, \
                 tc.tile_pool(name="sq", bufs=3) as sqp, \
                 tc.tile_pool(name="rowstg", bufs=4) as rowstg, \
                 tc.tile_pool(name="pss", bufs=4, space="PSUM") as pssp, \
                 tc.tile_pool(name="pk", bufs=2, space="PSUM") as pkp:
                ss_ps = [pssp.tile([1, NS], F32, tag="ss", name=f"ss_ps{i}")
                         for i in range(S // NS)]
                for m in range(DK):
                    wkcol = wkp.tile([P, DK, P], F32R, tag="wkcol")
                    eng = nc.sync if m % 2 == 0 else nc.gpsimd
                    eng.dma_start(
                        out=wkcol[:],
                        in_=wkT[:, m * P:(m + 1) * P].rearrange(
                            "(kk p) n -> p kk n", p=P))
                    for nn in range(S // NS):
                        ps = pkp.tile([P, NS], F32, tag="pk")
                        for kk in range(DK):
                            nc.tensor.matmul(
                                ps[:], wkcol[:, kk, :],
                                xts(kk, slice(nn * NS, (nn + 1) * NS)),
                                start=(kk == 0), stop=(kk == DK - 1))
                        nc.scalar.activation(
                            kt_sb[:, m, nn * NS:(nn + 1) * NS], ps[:],
                            AF.Identity, bias=bkc[:, m:m + 1])
                        sqt = sqp.tile([P, NS], F32R, tag="sq")
                        nc.scalar.activation(
                            sqt[:], kt_sb[:, m, nn * NS:(nn + 1) * NS],
                            AF.Square)
                        nc.tensor.matmul(
                            ss_ps[nn][:], ones_sb[:], sqt[:],
                            start=(m == 0), stop=(m == DK - 1))
                # rk chain: no PE instructions; hides under the Q phase
                for nn in range(S // NS):
                    rk_row = rowstg.tile([1, NS], F32, tag="rkrow")
                    nc.scalar.activation(rk_row[:], ss_ps[nn][:], AF.Sqrt)
                    nc.vector.reciprocal(rk_row[:], rk_row[:])
                    nc.scalar.dma_start(
                        out=rk_scr[0:1, nn * NS:(nn + 1) * NS], in_=rk_row[:])
                rk_flat = rk_scr[0:1, :]
                nc.scalar.dma_start(
                    out=rk_col[:],
                    in_=bass.AP(tensor=rk_flat.tensor, offset=rk_flat.offset,
                                ap=[[1, P], [P, SK]]))

            # ======== Q projection (feature-major, own rows = xT cols
            # [0, QROWS)) ========
            with tc.tile_pool(name="wq", bufs=4) as wqp, \
                 tc.tile_pool(name="sqq", bufs=3) as sqqp, \
                 tc.tile_pool(name="rowstgq", bufs=2) as rowstgq, \
                 tc.tile_pool(name="rqb", bufs=1) as rqbp, \
                 tc.tile_pool(name="pssq", bufs=2, space="PSUM") as pssqp, \
                 tc.tile_pool(name="pq", bufs=2, space="PSUM") as pqp:
                ssq_ps = [pssqp.tile([1, NS], F32, tag="ssq",
                                     name=f"ssq_ps{i}")
                          for i in range(QS2)]
                for m in range(DK):
                    wqcol = wqp.tile([P, DK, P], F32R, tag="wqcol")
                    eng = nc.sync if m % 2 == 0 else nc.gpsimd
                    eng.dma_start(
                        out=wqcol[:],
                        in_=wqT[:, m * P:(m + 1) * P].rearrange(
                            "(kk p) n -> p kk n", p=P))
                    for nn in range(QS2):
                        ps = pqp.tile([P, NS], F32, tag="pq")
                        for kk in range(DK):
                            nc.tensor.matmul(
                                ps[:], wqcol[:, kk, :],
                                xts(kk, slice(nn * NS, (nn + 1) * NS)),
                                start=(kk == 0), stop=(kk == DK - 1))
                        nc.scalar.activation(
                            qt_sb[:, m, nn * NS:(nn + 1) * NS], ps[:],
                            AF.Identity, bias=bqc[:, m:m + 1])
                        sqt = sqqp.tile([P, NS], F32R, tag="sqq")
                        nc.scalar.activation(
                            sqt[:], qt_sb[:, m, nn * NS:(nn + 1) * NS],
                            AF.Square)
                        nc.tensor.matmul(
                            ssq_ps[nn][:], ones_sb[:], sqt[:],
                            start=(m == 0), stop=(m == DK - 1))
                # rq chain (no PE): broadcast 1/||Q_row|| across partitions
                # via DRAM roundtrip; the multiplies ride on DVE during V.
                for nn in range(QS2):
                    rq_row = rowstgq.tile([1, NS], F32, tag="rqrow")
                    nc.scalar.activation(rq_row[:], ssq_ps[nn][:], AF.Sqrt)
                    nc.vector.reciprocal(rq_row[:], rq_row[:])
                    rq_rowr = rowstgq.tile([1, NS], F32R, tag="rqrowr")
                    nc.scalar.copy(rq_rowr[:], rq_row[:])
                    nc.scalar.dma_start(
                        out=rq_scr[0:1, nn * NS:(nn + 1) * NS],
                        in_=rq_rowr[:])
                rqb = rqbp.tile([P, QROWS], F32R)
                nc.gpsimd.dma_start(
                    out=rqb[:], in_=bcast_ap(rq_scr[0:1, :], QROWS))
                for m in range(DK):
                    for nn in range(QS2):
                        sl = slice(nn * NS, (nn + 1) * NS)
                        nc.vector.tensor_mul(
                            qt_sb[:, m, sl], qt_sb[:, m, sl], rqb[:, sl])

            # ======== V projection (natural) -> DRAM scratch ========
            with tc.tile_pool(name="wvh", bufs=1) as wvhp, \
                 tc.tile_pool(name="bvp", bufs=1) as bvp, \
                 tc.tile_pool(name="vstg", bufs=4) as vstg, \
                 tc.tile_pool(name="pv", bufs=6, space="PSUM") as pv:
                bvb = bvp.tile([P, D], F32)
                nc.gpsimd.dma_start(out=bvb[:],
                                    in_=bcast_ap(bv_row[0:1, :], D))
                for nn in range(D // NS):
                    wvh = wvhp.tile([P, DK, NS], F32R, tag="wvh")
                    for kk in range(DK):
                        nc.sync.dma_start(
                            out=wvh[:, kk, :],
                            in_=wvT[kk * P:(kk + 1) * P,
                                    nn * NS:(nn + 1) * NS])
                    for g in range(SK // VG):
                        pss_v = [pv.tile([P, NS], F32, tag="pv",
                                         name=f"pv_{nn}_{g}_{i}")
                                 for i in range(VG)]
                        for kk in range(DK):
                            for i in range(VG):
                                rm = g * VG + i
                                nc.tensor.matmul(
                                    pss_v[i][:],
                                    xts(kk, slice(rm * P, (rm + 1) * P)),
                                    wvh[:, kk, :],
                                    start=(kk == 0), stop=(kk == DK - 1))
                        for i in range(VG):
                            rm = g * VG + i
                            vs = vstg.tile([P, NS], F32R, tag="vs")
                            nc.vector.scalar_tensor_tensor(
                                out=vs[:], in0=pss_v[i][:], scalar=1.0,
                                in1=bvb[:, nn * NS:(nn + 1) * NS],
                                op0=ALU.mult, op1=ALU.add)
                            nc.gpsimd.dma_start(
                                out=v_scr[rm * P:(rm + 1) * P,
                                          nn * NS:(nn + 1) * NS],
                                in_=vs[:])
            xtbp.release()
            xtap.release()

            # ======== attention ========
            with tc.tile_pool(name="pt", bufs=1) as ptp, \
                 tc.tile_pool(name="vcol", bufs=3) as vcolp, \
                 tc.tile_pool(name="rsb", bufs=2) as rsbp, \
                 tc.tile_pool(name="rowstgs", bufs=2) as rowstgs, \
                 tc.tile_pool(name="ostg", bufs=4) as ostgp, \
                 tc.tile_pool(name="psim", bufs=2, space="PSUM") as psimp, \
                 tc.tile_pool(name="psums", bufs=2, space="PSUM") as psmsp, \
                 tc.tile_pool(name="po", bufs=3, space="PSUM") as pop:
                for qs in range(QS2):
                    qsl = slice(qs * NS, (qs + 1) * NS)
                    pt_sb = ptp.tile([P, SK, NS], F32R, tag="pt")
                    for kt in range(SK):
                        ps = psimp.tile([P, NS], F32, tag="psim")
                        for kk in range(DK):
                            nc.tensor.matmul(
                                ps[:], kt_sb[:, kk, kt * P:(kt + 1) * P],
                                qt_sb[:, kk, qsl],
                                start=(kk == 0), stop=(kk == DK - 1))
                        nc.scalar.activation(
                            pt_sb[:, kt, :], ps[:], AF.Exp,
                            scale=rk_col[:, kt:kt + 1])
                    ps_s = psmsp.tile([1, NS], F32, tag="ps_s")
                    for kt in range(SK):
                        nc.tensor.matmul(
                            ps_s[:], ones_sb[:], pt_sb[:, kt, :],
                            start=(kt == 0), stop=(kt == SK - 1))
                    # 1/sum chain, no PE instructions (DMA broadcast)
                    s_row = rowstgs.tile([1, NS], F32, tag="srow")
                    nc.vector.reciprocal(s_row[:], ps_s[:])
                    s_rowr = rowstgs.tile([1, NS], F32R, tag="srowr")
                    nc.scalar.copy(s_rowr[:], s_row[:])
                    nc.scalar.dma_start(
                        out=rs_scr[0:1, qs * NS:(qs + 1) * NS], in_=s_rowr[:])
                    rsb = rsbp.tile([P, NS], F32R, tag="rsb")
                    nc.gpsimd.dma_start(
                        out=rsb[:],
                        in_=bcast_ap(rs_scr[0:1, qs * NS:(qs + 1) * NS], NS))
                    for m in range(DK):
                        vc = vcolp.tile([P, SK, P], F32R, tag="vc")
                        eng = nc.sync if m % 2 == 0 else nc.gpsimd
                        eng.dma_start(
                            out=vc[:],
                            in_=v_scr[:, m * P:(m + 1) * P].rearrange(
                                "(kt p) d -> p kt d", p=P))
                        po_t = pop.tile([P, NS], F32, tag="po")
                        for kt in range(SK):
                            nc.tensor.matmul(
                                po_t[:], vc[:, kt, :], pt_sb[:, kt, :],
                                start=(kt == 0), stop=(kt == SK - 1))
                        os_t = ostgp.tile([P, NS], F32R, tag="ostg")
                        nc.vector.tensor_mul(os_t[:], po_t[:], rsb[:])
                        nc.gpsimd.dma_start(
                            out=ot_scr[qs][m * P:(m + 1) * P, :], in_=os_t[:])
            ktp.release()

            # ======== MLP ========
            # attention output read back into qt_sb (same shape/dtype; the
            # sim matmuls are its last readers).
            ot_rd = qt_sb
            for nn in range(QS2):
                for kk in range(DK):
                    eng = nc.sync if kk % 2 == 0 else nc.gpsimd
                    eng.dma_start(
                        out=ot_rd[:, kk, nn * NS:(nn + 1) * NS],
                        in_=ot_scr[nn][kk * P:(kk + 1) * P, :])
            h2ap = tc.alloc_tile_pool(name="h2a", bufs=1)
            h2bp = tc.alloc_tile_pool(name="h2b", bufs=1)
            h2cp = tc.alloc_tile_pool(name="h2c", bufs=1)
            h2dp = tc.alloc_tile_pool(name="h2d", bufs=1)
            h2_parts = [
                h2ap.tile([P, HK // 4, QROWS], F32R, name="h2a_sb"),
                h2bp.tile([P, HK // 4, QROWS], F32R, name="h2b_sb"),
                h2cp.tile([P, HK // 4, QROWS], F32R, name="h2c_sb"),
                h2dp.tile([P, HK // 4, QROWS], F32R, name="h2d_sb"),
            ]

            def h2s(kt, sl):
                return h2_parts[kt // (HK // 4)][:, kt % (HK // 4), sl]

            h1tp = tc.alloc_tile_pool(name="h1t", bufs=1)
            h1_sb = h1tp.tile([P, DK, QROWS], F32R)

            with tc.tile_pool(name="w1", bufs=2) as w1p, \
                 tc.tile_pool(name="p1", bufs=3, space="PSUM") as p1p:
                for m in range(DK):
                    w1col = w1p.tile([P, DK, P], F32R, tag="w1col")
                    eng = nc.sync if m % 2 == 0 else nc.gpsimd
                    eng.dma_start(
                        out=w1col[:],
                        in_=w1T[:, m * P:(m + 1) * P].rearrange(
                            "(kk p) n -> p kk n", p=P))
                    for nn in range(QS2):
                        ps = p1p.tile([P, NS], F32, tag="p1")
                        for kk in range(DK):
                            nc.tensor.matmul(
                                ps[:], w1col[:, kk, :],
                                ot_rd[:, kk, nn * NS:(nn + 1) * NS],
                                start=(kk == 0), stop=(kk == DK - 1))
                        nc.scalar.activation(
                            h1_sb[:, m, nn * NS:(nn + 1) * NS], ps[:],
                            AF.Relu, bias=b1c[:, m:m + 1])

            with tc.tile_pool(name="w2", bufs=3) as w2p, \
                 tc.tile_pool(name="p2", bufs=3, space="PSUM") as p2p:
                for kt in range(HK):
                    w2col = w2p.tile([P, DK, P], F32R, tag="w2col")
                    eng = nc.sync if kt % 2 == 0 else nc.gpsimd
                    eng.dma_start(
                        out=w2col[:],
                        in_=w2T[:, kt * P:(kt + 1) * P].rearrange(
                            "(kk p) n -> p kk n", p=P))
                    for nn in range(QS2):
                        ps = p2p.tile([P, NS], F32, tag="p2")
                        for kk in range(DK):
                            nc.tensor.matmul(
                                ps[:], w2col[:, kk, :],
                                h1_sb[:, kk, nn * NS:(nn + 1) * NS],
                                start=(kk == 0), stop=(kk == DK - 1))
                        nc.scalar.activation(
                            h2s(kt, slice(nn * NS, (nn + 1) * NS)), ps[:],
                            AF.Relu, bias=b2c[:, kt:kt + 1])
            h1tp.release()

            with tc.tile_pool(name="w3", bufs=3) as w3p, \
                 tc.tile_pool(name="ostg3", bufs=4) as ostg3, \
                 tc.tile_pool(name="b3p", bufs=1) as b3p, \
                 tc.tile_pool(name="p3", bufs=8, space="PSUM") as p3p:
                for nn in range(D // NS):
                    b3rn = b3p.tile([1, NS], F32R, tag="b3rn")
                    nc.sync.dma_start(
                        out=b3rn[:], in_=b3_row[0:1, nn * NS:(nn + 1) * NS])
                    h3ps = [p3p.tile([P, NS], F32, tag="h3",
                                     name=f"h3ps_{nn}_{i}")
                            for i in range(QT8)]
                    for t in range(QT8):
                        nc.tensor.matmul(h3ps[t][:], onesr_sb[:], b3rn[:],
                                         start=True, stop=False)
                    for kt2 in range(HK // 2):
                        w3t = w3p.tile([P, 2, NS], F32R, tag="w3t")
                        eng = nc.sync if kt2 % 2 == 0 else nc.gpsimd
                        eng.dma_start(
                            out=w3t[:],
                            in_=w3T[kt2 * 2 * P:(kt2 + 1) * 2 * P,
                                    nn * NS:(nn + 1) * NS].rearrange(
                                "(two p) n -> p two n", p=P))
                        for two in range(2):
                            kt = kt2 * 2 + two
                            for t in range(QT8):
                                nc.tensor.matmul(
                                    h3ps[t][:],
                                    h2s(kt, slice(t * P, (t + 1) * P)),
                                    w3t[:, two, :],
                                    start=False, stop=(kt == HK - 1))
                    for t in range(QT8):
                        os_t = ostg3.tile([P, NS], F32, tag="ostg3")
                        nc.vector.tensor_copy(out=os_t[:], in_=h3ps[t][:])
                        nc.gpsimd.dma_start(
                            out=out[t * P:(t + 1) * P, nn * NS:(nn + 1) * NS],
                            in_=os_t[:])
            h2dp.release()
            h2cp.release()
            h2bp.release()
            h2ap.release()
            xtbp_dummy = None  # placeholder to keep structure clear
            ktp_dummy = None
            qtp.release()

    nc.compile()
    return nc


def _get_built():
    global _BUILT
    if _BUILT is None:
        _BUILT = _build()
    return _BUILT


def _host_prep(inputs):
    f32 = np.float32
    x = np.asarray(inputs["x"], f32)
    shared = {
        "wqT": np.ascontiguousarray(np.asarray(inputs["Wq"], f32).T),
        "wkT": np.ascontiguousarray(np.asarray(inputs["Wk"], f32).T),
        "wvT": np.ascontiguousarray(np.asarray(inputs["Wv"], f32).T),
        "w1T": np.ascontiguousarray(np.asarray(inputs["W1"], f32).T),
        "w2T": np.ascontiguousarray(np.asarray(inputs["W2"], f32).T),
        "w3T": np.ascontiguousarray(np.asarray(inputs["W3"], f32).T),
        "bq_col": np.ascontiguousarray(
            np.asarray(inputs["bq"], f32).reshape(D // P, P).T),
        "bk_col": np.ascontiguousarray(
            np.asarray(inputs["bk"], f32).reshape(D // P, P).T),
        "b1_col": np.ascontiguousarray(
            np.asarray(inputs["b1"], f32).reshape(D // P, P).T),
        "b2_col": np.ascontiguousarray(
            np.asarray(inputs["b2"], f32).reshape(H // P, P).T),
        "bv_row": np.asarray(inputs["bv"], f32).reshape(1, D),
        "b3_row": np.asarray(inputs["b3"], f32).reshape(1, D),
        "ones_col": np.ones((P, 1), f32),
        "ones_row": np.ones((1, P), f32),
    }
    in_maps = []
    for c in range(N_CORES):
        b, h = c // 2, c % 2
        m = dict(shared)
        if h == 0:
            xrot = x[b]
        else:
            xrot = np.concatenate([x[b, QROWS:], x[b, :QROWS]], axis=0)
        m["xT"] = np.ascontiguousarray(xrot.T)
        in_maps.append(m)
    return in_maps


def run_kernel(inputs, trace=False):
    """Returns (output [B,S,D] f32, exec_time_ns or None)."""
    from concourse.bass_utils import run_bass_kernel_spmd

    if trace:
        _install_ntff_hook()
    nc = _get_built()
    in_maps = _host_prep(inputs)
    res = run_bass_kernel_spmd(
        nc, in_maps, core_ids=list(range(N_CORES)), trace=trace)
    global _LAST_INSTS
    if res.instructions_and_trace is not None:
        _LAST_INSTS = res.instructions_and_trace[0]
    outp = np.empty((B, S, D), np.float32)
    for c in range(N_CORES):
        b, h = c // 2, c % 2
        outp[b, h * QROWS:(h + 1) * QROWS, :] = res.results[c]["out"]
    return outp, res.exec_time_ns


def kernel(**inputs):
    return run_kernel(inputs, trace=False)[0]


def _install_ntff_hook():
    """Register the axon NTFF profiling hook (used only when trace=True)."""
    import sys
    import types

    if "antenv.axon_hooks" in sys.modules:
        return
    try:
        import antenv
        from trn_agent_boot.trn_boot import _ntff_profile_via_ctypes
    except ImportError:
        return
    hooks = types.ModuleType("antenv.axon_hooks")
    _h = [_ntff_profile_via_ctypes("/opt/axon/libaxon_pjrt.so")]
    hooks.set_axon_ntff_profile_hook = lambda h: _h.__setitem__(0, h)
    hooks.get_axon_ntff_profile_hook = lambda: _h[0]
    sys.modules["antenv.axon_hooks"] = hooks
    antenv.axon_hooks = hooks


# revision 18
# speedup vs baseline: 1.0072x; 1.0072x over previous
"""Trainium2 Bass kernel for nn_Attention_28604482191653.

Reference computation (B=4, S=2048, D=1024, H=4096, fp32):
    Q = x@Wq.T+bq; K = x@Wk.T+bk; V = x@Wv.T+bv     (per batch b)
    Q,K l2-normalized along features; sim = Q@K.T; attn = softmax(sim)
    out = attn@V; mlp: relu(out@W1.T+b1) -> relu(@W2.T+b2) -> @W3.T+b3

Sharding: 8 cores = (batch b, sequence half h); core c handles b=c//2,
query rows [h*1024, (h+1)*1024). K/V are recomputed per core pair (no
collectives). The host rotates x[b]'s rows so each core's own query rows
are always columns [0, 1024) of xT — softmax/PV are invariant to key
order, so K/V just use the rotated order consistently.

On-chip activations are feature-major ("T" = [features, rows]) so biases
and softmax scales land on the partition dimension; matmul dtype is
float32r (bf16-class PE rate, ~tf32 accuracy, fp32 PSUM accumulate).

Phases are ordered K -> Q -> V -> attention -> MLP so that each
normalization chain (1/||row|| via ones-vector matmuls + sqrt/recip +
DMA partition-broadcast, no PE instructions) hides under the next
matmul phase. DMA triggers alternate between the SP ("sync") HW-DGE
queue and the GpSimd SW-DGE queue; ACT-issued DMAs are avoided where
the ACT stream is busy (triggers are serialized with compute per
engine).

L2-normalize eps: reference uses max(||row||, 1e-12); row norms here are
~sqrt(D) >> eps with probability 1, so plain rsqrt matches.
"""

import numpy as np

B, S, D, H = 4, 2048, 1024, 4096
P = 128
NS = 512          # matmul moving-dim slice
QROWS = S // 2    # query rows per core
N_CORES = 8

_BUILT = None
_LAST_INSTS = None


def _build():
    import concourse.bass as bass
    import concourse.tile as tile
    from concourse import bacc, mybir

    F32 = mybir.dt.float32
    F32R = mybir.dt.float32r
    ALU = mybir.AluOpType
    AF = mybir.ActivationFunctionType

    nc = bacc.Bacc("TRN2", target_bir_lowering=False, debug=False)

    # ---- I/O ----
    xT = nc.dram_tensor("xT", [D, S], F32R, kind="ExternalInput")
    wqT = nc.dram_tensor("wqT", [D, D], F32R, kind="ExternalInput")
    wkT = nc.dram_tensor("wkT", [D, D], F32R, kind="ExternalInput")
    wvT = nc.dram_tensor("wvT", [D, D], F32R, kind="ExternalInput")
    w1T = nc.dram_tensor("w1T", [D, D], F32R, kind="ExternalInput")
    w2T = nc.dram_tensor("w2T", [D, H], F32R, kind="ExternalInput")
    w3T = nc.dram_tensor("w3T", [H, D], F32R, kind="ExternalInput")
    bq_col = nc.dram_tensor("bq_col", [P, D // P], F32, kind="ExternalInput")
    bk_col = nc.dram_tensor("bk_col", [P, D // P], F32, kind="ExternalInput")
    b1_col = nc.dram_tensor("b1_col", [P, D // P], F32, kind="ExternalInput")
    b2_col = nc.dram_tensor("b2_col", [P, H // P], F32, kind="ExternalInput")
    bv_row = nc.dram_tensor("bv_row", [1, D], F32, kind="ExternalInput")
    b3_row = nc.dram_tensor("b3_row", [1, D], F32R, kind="ExternalInput")
    ones_col = nc.dram_tensor("ones_col", [P, 1], F32R, kind="ExternalInput")
    ones_row = nc.dram_tensor("ones_row", [1, P], F32R, kind="ExternalInput")
    out = nc.dram_tensor("out", [QROWS, D], F32, kind="ExternalOutput")

    def bcast_ap(sliced, n):
        """AP replicating a [1, n] DRAM slice across all 128 partitions."""
        return bass.AP(tensor=sliced.tensor, offset=sliced.offset,
                       ap=[[0, P], [1, n]])

    DK = D // P    # 8 feature tiles of d_model
    SK = S // P    # 16 key-position tiles
    HK = H // P    # 32 hidden tiles
    QT8 = QROWS // P
    QS2 = QROWS // NS
    VG = 4         # V row-tiles per psum group

    with tile.TileContext(nc, pool_alloc_mode="queue") as tc:
        with tc.tile_pool(name="dram", bufs=1, space="DRAM") as dram, \
             tc.tile_pool(name="const", bufs=1) as const:
            v_scr = dram.tile([S, D], F32R)
            ot_scr = [dram.tile([D, NS], F32R, name=f"ot_scr{i}")
                      for i in range(QS2)]
            rk_scr = dram.tile([1, S], F32)
            rq_scr = dram.tile([1, QROWS], F32R)
            rs_scr = dram.tile([1, QROWS], F32R)

            ones_sb = const.tile([P, 1], F32R)
            nc.sync.dma_start(out=ones_sb[:], in_=ones_col[:, :])
            onesr_sb = const.tile([1, P], F32R)
            nc.sync.dma_start(out=onesr_sb[:], in_=ones_row[:, :])
            bqc = const.tile([P, DK], F32)
            nc.sync.dma_start(out=bqc[:], in_=bq_col[:, :])
            bkc = const.tile([P, DK], F32)
            nc.sync.dma_start(out=bkc[:], in_=bk_col[:, :])
            b1c = const.tile([P, DK], F32)
            nc.sync.dma_start(out=b1c[:], in_=b1_col[:, :])
            b2c = const.tile([P, HK], F32)
            nc.sync.dma_start(out=b2c[:], in_=b2_col[:, :])
            rk_col = const.tile([P, SK], F32)

            qtp = tc.alloc_tile_pool(name="qt", bufs=1)
            qt_sb = qtp.tile([P, DK, QROWS], F32R)
            ktp = tc.alloc_tile_pool(name="kt", bufs=1)
            kt_sb = ktp.tile([P, DK, S], F32R)
            xtap = tc.alloc_tile_pool(name="xta", bufs=1)
            xtbp = tc.alloc_tile_pool(name="xtb", bufs=1)
            xta_sb = xtap.tile([P, DK // 2, S], F32R)
            xtb_sb = xtbp.tile([P, DK // 2, S], F32R)

            def xts(kk, sl):
                if kk < DK // 2:
                    return xta_sb[:, kk, sl]
                return xtb_sb[:, kk - DK // 2, sl]

            for nn in range(S // NS):
                for kk in range(DK):
                    nc.sync.dma_start(
                        out=xts(kk, slice(nn * NS, (nn + 1) * NS)),
                        in_=xT[kk * P:(kk + 1) * P, nn * NS:(nn + 1) * NS])

            # ======== K projection (feature-major, resident) ========
            with tc.tile_pool(name="wk", bufs=4) as wkp, \
                 tc.tile_pool(name="sq", bufs=3) as sqp, \
                 tc.tile_pool(name="rowstg", bufs=4) as rowstg, \
                 tc.tile_pool(name="pss", bufs=4, space="PSUM") as pssp, \
                 tc.tile_pool(name="pk", bufs=2, space="PSUM") as pkp:
                ss_ps = [pssp.tile([1, NS], F32, tag="ss", name=f"ss_ps{i}")
                         for i in range(S // NS)]
                for m in range(DK):
                    wkcol = wkp.tile([P, DK, P], F32R, tag="wkcol")
                    nc.scalar.dma_start(
                        out=wkcol[:],
                        in_=wkT[:, m * P:(m + 1) * P].rearrange(
                            "(kk p) n -> p kk n", p=P))
                    for nn in range(S // NS):
                        ps = pkp.tile([P, NS], F32, tag="pk")
                        for kk in range(DK):
                            nc.tensor.matmul(
                                ps[:], wkcol[:, kk, :],
                                xts(kk, slice(nn * NS, (nn + 1) * NS)),
                                start=(kk == 0), stop=(kk == DK - 1))
                        nc.scalar.activation(
                            kt_sb[:, m, nn * NS:(nn + 1) * NS], ps[:],
                            AF.Identity, bias=bkc[:, m:m + 1])
                        sqt = sqp.tile([P, NS], F32R, tag="sq")
                        nc.vector.tensor_mul(
                            sqt[:], kt_sb[:, m, nn * NS:(nn + 1) * NS],
                            kt_sb[:, m, nn * NS:(nn + 1) * NS])
                        nc.tensor.matmul(
                            ss_ps[nn][:], ones_sb[:], sqt[:],
                            start=(m == 0), stop=(m == DK - 1))
                # rk chain: no PE instructions; hides under the Q phase
                for nn in range(S // NS):
                    rk_row = rowstg.tile([1, NS], F32, tag="rkrow")
                    nc.scalar.activation(rk_row[:], ss_ps[nn][:], AF.Sqrt)
                    nc.vector.reciprocal(rk_row[:], rk_row[:])
                    nc.scalar.dma_start(
                        out=rk_scr[0:1, nn * NS:(nn + 1) * NS], in_=rk_row[:])
                rk_flat = rk_scr[0:1, :]
                nc.scalar.dma_start(
                    out=rk_col[:],
                    in_=bass.AP(tensor=rk_flat.tensor, offset=rk_flat.offset,
                                ap=[[1, P], [P, SK]]))

            # ======== Q projection (feature-major, own rows = xT cols
            # [0, QROWS)) ========
            with tc.tile_pool(name="wq", bufs=4) as wqp, \
                 tc.tile_pool(name="sqq", bufs=3) as sqqp, \
                 tc.tile_pool(name="rowstgq", bufs=2) as rowstgq, \
                 tc.tile_pool(name="rqb", bufs=1) as rqbp, \
                 tc.tile_pool(name="pssq", bufs=2, space="PSUM") as pssqp, \
                 tc.tile_pool(name="pq", bufs=2, space="PSUM") as pqp:
                ssq_ps = [pssqp.tile([1, NS], F32, tag="ssq",
                                     name=f"ssq_ps{i}")
                          for i in range(QS2)]
                for m in range(DK):
                    wqcol = wqp.tile([P, DK, P], F32R, tag="wqcol")
                    nc.scalar.dma_start(
                        out=wqcol[:],
                        in_=wqT[:, m * P:(m + 1) * P].rearrange(
                            "(kk p) n -> p kk n", p=P))
                    for nn in range(QS2):
                        ps = pqp.tile([P, NS], F32, tag="pq")
                        for kk in range(DK):
                            nc.tensor.matmul(
                                ps[:], wqcol[:, kk, :],
                                xts(kk, slice(nn * NS, (nn + 1) * NS)),
                                start=(kk == 0), stop=(kk == DK - 1))
                        nc.scalar.activation(
                            qt_sb[:, m, nn * NS:(nn + 1) * NS], ps[:],
                            AF.Identity, bias=bqc[:, m:m + 1])
                        sqt = sqqp.tile([P, NS], F32R, tag="sqq")
                        nc.vector.tensor_mul(
                            sqt[:], qt_sb[:, m, nn * NS:(nn + 1) * NS],
                            qt_sb[:, m, nn * NS:(nn + 1) * NS])
                        nc.tensor.matmul(
                            ssq_ps[nn][:], ones_sb[:], sqt[:],
                            start=(m == 0), stop=(m == DK - 1))
                # rq chain (no PE): broadcast 1/||Q_row|| across partitions
                # via DRAM roundtrip; the multiplies ride on DVE during V.
                for nn in range(QS2):
                    rq_row = rowstgq.tile([1, NS], F32, tag="rqrow")
                    nc.scalar.activation(rq_row[:], ssq_ps[nn][:], AF.Sqrt)
                    nc.vector.reciprocal(rq_row[:], rq_row[:])
                    rq_rowr = rowstgq.tile([1, NS], F32R, tag="rqrowr")
                    nc.scalar.copy(rq_rowr[:], rq_row[:])
                    nc.scalar.dma_start(
                        out=rq_scr[0:1, nn * NS:(nn + 1) * NS],
                        in_=rq_rowr[:])
                rqb = rqbp.tile([P, QROWS], F32R)
                nc.gpsimd.dma_start(
                    out=rqb[:], in_=bcast_ap(rq_scr[0:1, :], QROWS))
                for m in range(DK):
                    for nn in range(QS2):
                        sl = slice(nn * NS, (nn + 1) * NS)
                        nc.vector.tensor_mul(
                            qt_sb[:, m, sl], qt_sb[:, m, sl], rqb[:, sl])

            # ======== V projection (natural) -> DRAM scratch ========
            with tc.tile_pool(name="wvh", bufs=1) as wvhp, \
                 tc.tile_pool(name="bvp", bufs=1) as bvp, \
                 tc.tile_pool(name="vstg", bufs=4) as vstg, \
                 tc.tile_pool(name="pv", bufs=6, space="PSUM") as pv:
                bvb = bvp.tile([P, D], F32)
                nc.gpsimd.dma_start(out=bvb[:],
                                    in_=bcast_ap(bv_row[0:1, :], D))
                for nn in range(D // NS):
                    wvh = wvhp.tile([P, DK, NS], F32R, tag="wvh")
                    for kk in range(DK):
                        nc.sync.dma_start(
                            out=wvh[:, kk, :],
                            in_=wvT[kk * P:(kk + 1) * P,
                                    nn * NS:(nn + 1) * NS])
                    for g in range(SK // VG):
                        pss_v = [pv.tile([P, NS], F32, tag="pv",
                                         name=f"pv_{nn}_{g}_{i}")
                                 for i in range(VG)]
                        for kk in range(DK):
                            for i in range(VG):
                                rm = g * VG + i
                                nc.tensor.matmul(
                                    pss_v[i][:],
                                    xts(kk, slice(rm * P, (rm + 1) * P)),
                                    wvh[:, kk, :],
                                    start=(kk == 0), stop=(kk == DK - 1))
                        for i in range(VG):
                            rm = g * VG + i
                            vs = vstg.tile([P, NS], F32R, tag="vs")
                            nc.vector.scalar_tensor_tensor(
                                out=vs[:], in0=pss_v[i][:], scalar=1.0,
                                in1=bvb[:, nn * NS:(nn + 1) * NS],
                                op0=ALU.mult, op1=ALU.add)
                            weng = nc.sync if rm % 2 == 0 else nc.gpsimd
                            weng.dma_start(
                                out=v_scr[rm * P:(rm + 1) * P,
                                          nn * NS:(nn + 1) * NS],
                                in_=vs[:])
            xtbp.release()
            xtap.release()

            # ======== attention ========
            with tc.tile_pool(name="pt", bufs=1) as ptp, \
                 tc.tile_pool(name="vcol", bufs=3) as vcolp, \
                 tc.tile_pool(name="rsb", bufs=2) as rsbp, \
                 tc.tile_pool(name="rowstgs", bufs=2) as rowstgs, \
                 tc.tile_pool(name="ostg", bufs=4) as ostgp, \
                 tc.tile_pool(name="psim", bufs=2, space="PSUM") as psimp, \
                 tc.tile_pool(name="psums", bufs=2, space="PSUM") as psmsp, \
                 tc.tile_pool(name="po", bufs=3, space="PSUM") as pop:
                for qs in range(QS2):
                    qsl = slice(qs * NS, (qs + 1) * NS)
                    pt_sb = ptp.tile([P, SK, NS], F32R, tag="pt")
                    for kt in range(SK):
                        ps = psimp.tile([P, NS], F32, tag="psim")
                        for kk in range(DK):
                            nc.tensor.matmul(
                                ps[:], kt_sb[:, kk, kt * P:(kt + 1) * P],
                                qt_sb[:, kk, qsl],
                                start=(kk == 0), stop=(kk == DK - 1))
                        nc.scalar.activation(
                            pt_sb[:, kt, :], ps[:], AF.Exp,
                            scale=rk_col[:, kt:kt + 1])
                    ps_s = psmsp.tile([1, NS], F32, tag="ps_s")
                    for kt in range(SK):
                        nc.tensor.matmul(
                            ps_s[:], ones_sb[:], pt_sb[:, kt, :],
                            start=(kt == 0), stop=(kt == SK - 1))
                    # 1/sum chain, no PE instructions (DMA broadcast)
                    s_row = rowstgs.tile([1, NS], F32, tag="srow")
                    nc.vector.reciprocal(s_row[:], ps_s[:])
                    s_rowr = rowstgs.tile([1, NS], F32R, tag="srowr")
                    nc.scalar.copy(s_rowr[:], s_row[:])
                    nc.scalar.dma_start(
                        out=rs_scr[0:1, qs * NS:(qs + 1) * NS], in_=s_rowr[:])
                    rsb = rsbp.tile([P, NS], F32R, tag="rsb")
                    nc.gpsimd.dma_start(
                        out=rsb[:],
                        in_=bcast_ap(rs_scr[0:1, qs * NS:(qs + 1) * NS], NS))
                    for m in range(DK):
                        vc = vcolp.tile([P, SK, P], F32R, tag="vc")
                        eng = nc.sync if m % 2 == 0 else nc.scalar
                        eng.dma_start(
                            out=vc[:],
                            in_=v_scr[:, m * P:(m + 1) * P].rearrange(
                                "(kt p) d -> p kt d", p=P))
                        po_t = pop.tile([P, NS], F32, tag="po")
                        for kt in range(SK):
                            nc.tensor.matmul(
                                po_t[:], vc[:, kt, :], pt_sb[:, kt, :],
                                start=(kt == 0), stop=(kt == SK - 1))
                        os_t = ostgp.tile([P, NS], F32R, tag="ostg")
                        nc.vector.tensor_mul(os_t[:], po_t[:], rsb[:])
                        weng = nc.gpsimd if m % 2 == 0 else nc.scalar
                        weng.dma_start(
                            out=ot_scr[qs][m * P:(m + 1) * P, :], in_=os_t[:])
            ktp.release()

            # ======== MLP ========
            # attention output read back into qt_sb (same shape/dtype; the
            # sim matmuls are its last readers).
            ot_rd = qt_sb
            for nn in range(QS2):
                for kk in range(DK):
                    nc.sync.dma_start(
                        out=ot_rd[:, kk, nn * NS:(nn + 1) * NS],
                        in_=ot_scr[nn][kk * P:(kk + 1) * P, :])
            h2ap = tc.alloc_tile_pool(name="h2a", bufs=1)
            h2bp = tc.alloc_tile_pool(name="h2b", bufs=1)
            h2cp = tc.alloc_tile_pool(name="h2c", bufs=1)
            h2dp = tc.alloc_tile_pool(name="h2d", bufs=1)
            h2_parts = [
                h2ap.tile([P, HK // 4, QROWS], F32R, name="h2a_sb"),
                h2bp.tile([P, HK // 4, QROWS], F32R, name="h2b_sb"),
                h2cp.tile([P, HK // 4, QROWS], F32R, name="h2c_sb"),
                h2dp.tile([P, HK // 4, QROWS], F32R, name="h2d_sb"),
            ]

            def h2s(kt, sl):
                return h2_parts[kt // (HK // 4)][:, kt % (HK // 4), sl]

            h1tp = tc.alloc_tile_pool(name="h1t", bufs=1)
            h1_sb = h1tp.tile([P, DK, QROWS], F32R)

            with tc.tile_pool(name="w1", bufs=2) as w1p, \
                 tc.tile_pool(name="p1", bufs=3, space="PSUM") as p1p:
                for m in range(DK):
                    w1col = w1p.tile([P, DK, P], F32R, tag="w1col")
                    nc.scalar.dma_start(
                        out=w1col[:],
                        in_=w1T[:, m * P:(m + 1) * P].rearrange(
                            "(kk p) n -> p kk n", p=P))
                    for nn in range(QS2):
                        ps = p1p.tile([P, NS], F32, tag="p1")
                        for kk in range(DK):
                            nc.tensor.matmul(
                                ps[:], w1col[:, kk, :],
                                ot_rd[:, kk, nn * NS:(nn + 1) * NS],
                                start=(kk == 0), stop=(kk == DK - 1))
                        nc.scalar.activation(
                            h1_sb[:, m, nn * NS:(nn + 1) * NS], ps[:],
                            AF.Relu, bias=b1c[:, m:m + 1])

            with tc.tile_pool(name="w2", bufs=3) as w2p, \
                 tc.tile_pool(name="p2", bufs=3, space="PSUM") as p2p:
                for kt in range(HK):
                    w2col = w2p.tile([P, DK, P], F32R, tag="w2col")
                    eng = nc.sync if kt % 2 == 0 else nc.gpsimd
                    eng.dma_start(
                        out=w2col[:],
                        in_=w2T[:, kt * P:(kt + 1) * P].rearrange(
                            "(kk p) n -> p kk n", p=P))
                    for nn in range(QS2):
                        ps = p2p.tile([P, NS], F32, tag="p2")
                        for kk in range(DK):
                            nc.tensor.matmul(
                                ps[:], w2col[:, kk, :],
                                h1_sb[:, kk, nn * NS:(nn + 1) * NS],
                                start=(kk == 0), stop=(kk == DK - 1))
                        nc.scalar.activation(
                            h2s(kt, slice(nn * NS, (nn + 1) * NS)), ps[:],
                            AF.Relu, bias=b2c[:, kt:kt + 1])
            h1tp.release()

            with tc.tile_pool(name="w3", bufs=3) as w3p, \
                 tc.tile_pool(name="ostg3", bufs=4) as ostg3, \
                 tc.tile_pool(name="b3p", bufs=1) as b3p, \
                 tc.tile_pool(name="p3", bufs=8, space="PSUM") as p3p:
                for nn in range(D // NS):
                    b3rn = b3p.tile([1, NS], F32R, tag="b3rn")
                    nc.sync.dma_start(
                        out=b3rn[:], in_=b3_row[0:1, nn * NS:(nn + 1) * NS])
                    h3ps = [p3p.tile([P, NS], F32, tag="h3",
                                     name=f"h3ps_{nn}_{i}")
                            for i in range(QT8)]
                    for t in range(QT8):
                        nc.tensor.matmul(h3ps[t][:], onesr_sb[:], b3rn[:],
                                         start=True, stop=False)
                    for kt2 in range(HK // 2):
                        w3t = w3p.tile([P, 2, NS], F32R, tag="w3t")
                        eng = nc.sync if kt2 % 2 == 0 else nc.gpsimd
                        eng.dma_start(
                            out=w3t[:],
                            in_=w3T[kt2 * 2 * P:(kt2 + 1) * 2 * P,
                                    nn * NS:(nn + 1) * NS].rearrange(
                                "(two p) n -> p two n", p=P))
                        for two in range(2):
                            kt = kt2 * 2 + two
                            for t in range(QT8):
                                nc.tensor.matmul(
                                    h3ps[t][:],
                                    h2s(kt, slice(t * P, (t + 1) * P)),
                                    w3t[:, two, :],
                                    start=False, stop=(kt == HK - 1))
                    for t in range(QT8):
                        os_t = ostg3.tile([P, NS], F32, tag="ostg3")
                        nc.vector.tensor_copy(out=os_t[:], in_=h3ps[t][:])
                        weng = nc.gpsimd if t % 2 == 0 else nc.scalar
                        weng.dma_start(
                            out=out[t * P:(t + 1) * P, nn * NS:(nn + 1) * NS],
                            in_=os_t[:])
            h2dp.release()
            h2cp.release()
            h2bp.release()
            h2ap.release()
            xtbp_dummy = None  # placeholder to keep structure clear
            ktp_dummy = None
            qtp.release()

    nc.compile()
    return nc


def _get_built():
    global _BUILT
    if _BUILT is None:
        _BUILT = _build()
    return _BUILT


def _host_prep(inputs):
    f32 = np.float32
    x = np.asarray(inputs["x"], f32)
    shared = {
        "wqT": np.ascontiguousarray(np.asarray(inputs["Wq"], f32).T),
        "wkT": np.ascontiguousarray(np.asarray(inputs["Wk"], f32).T),
        "wvT": np.ascontiguousarray(np.asarray(inputs["Wv"], f32).T),
        "w1T": np.ascontiguousarray(np.asarray(inputs["W1"], f32).T),
        "w2T": np.ascontiguousarray(np.asarray(inputs["W2"], f32).T),
        "w3T": np.ascontiguousarray(np.asarray(inputs["W3"], f32).T),
        "bq_col": np.ascontiguousarray(
            np.asarray(inputs["bq"], f32).reshape(D // P, P).T),
        "bk_col": np.ascontiguousarray(
            np.asarray(inputs["bk"], f32).reshape(D // P, P).T),
        "b1_col": np.ascontiguousarray(
            np.asarray(inputs["b1"], f32).reshape(D // P, P).T),
        "b2_col": np.ascontiguousarray(
            np.asarray(inputs["b2"], f32).reshape(H // P, P).T),
        "bv_row": np.asarray(inputs["bv"], f32).reshape(1, D),
        "b3_row": np.asarray(inputs["b3"], f32).reshape(1, D),
        "ones_col": np.ones((P, 1), f32),
        "ones_row": np.ones((1, P), f32),
    }
    in_maps = []
    for c in range(N_CORES):
        b, h = c // 2, c % 2
        m = dict(shared)
        if h == 0:
            xrot = x[b]
        else:
            xrot = np.concatenate([x[b, QROWS:], x[b, :QROWS]], axis=0)
        m["xT"] = np.ascontiguousarray(xrot.T)
        in_maps.append(m)
    return in_maps


def run_kernel(inputs, trace=False):
    """Returns (output [B,S,D] f32, exec_time_ns or None)."""
    from concourse.bass_utils import run_bass_kernel_spmd

    if trace:
        _install_ntff_hook()
    nc = _get_built()
    in_maps = _host_prep(inputs)
    res = run_bass_kernel_spmd(
        nc, in_maps, core_ids=list(range(N_CORES)), trace=trace)
    global _LAST_INSTS
    if res.instructions_and_trace is not None:
        _LAST_INSTS = res.instructions_and_trace[0]
    outp = np.empty((B, S, D), np.float32)
    for c in range(N_CORES):
        b, h = c // 2, c % 2
        outp[b, h * QROWS:(h + 1) * QROWS, :] = res.results[c]["out"]
    return outp, res.exec_time_ns


def kernel(**inputs):
    return run_kernel(inputs, trace=False)[0]


def _install_ntff_hook():
    """Register the axon NTFF profiling hook (used only when trace=True)."""
    import sys
    import types

    if "antenv.axon_hooks" in sys.modules:
        return
    try:
        import antenv
        from trn_agent_boot.trn_boot import _ntff_profile_via_ctypes
    except ImportError:
        return
    hooks = types.ModuleType("antenv.axon_hooks")
    _h = [_ntff_profile_via_ctypes("/opt/axon/libaxon_pjrt.so")]
    hooks.set_axon_ntff_profile_hook = lambda h: _h.__setitem__(0, h)
    hooks.get_axon_ntff_profile_hook = lambda: _h[0]
    sys.modules["antenv.axon_hooks"] = hooks
    antenv.axon_hooks = hooks
